# revision 1
# baseline (speedup 1.0000x reference)
"""Trainium2 Bass kernel for the CSD InfoNCE loss (nn_CSD_2d_55791625175673).

Strategy (data-parallel over batch B=8, one image per NeuronCore):
  * Host replicates the reference's threefry randomness + sampling index
    logic bit-exactly in numpy (tiny [B,H,W] control tensors only).
  * Each core streams its 3x8MB feature shard once, computing the 12
    per-class masked sums (the class means) as PE matmuls against host
    prepared one-hot/nv mask channels, and indirect-DMA-gathers its owned
    sampled feature rows into a 1664x128 contribution buffer.
  * One AllReduce(add) assembles the full sampled-feature matrix M on
    every core; each core then computes the InfoNCE log-softmax for its
    ~65 anchor rows and emits a partial scalar loss; host sums 8 scalars.
"""

import sys

import numpy as np

if "/opt/trn_rl_repo" not in sys.path:  # concourse toolchain
    sys.path.insert(0, "/opt/trn_rl_repo")

NUM_CLASS = 4
K = 512
TEMP = 0.1
B, D, H, W = 8, 128, 128, 128
N = B * H * W
NPIX = N // 8  # pixels per core (= H*W with one image per core)

NSAMP = K // NUM_CLASS  # 128 samples per class (K % NUM_CLASS == 0)
GRP = NSAMP + 1  # 129 rows per reference group (samples + mean)
NL = NUM_CLASS * GRP  # 516 anchor rows
NU = 2 * NUM_CLASS * GRP  # 1032 contrast rows
# physical M layout (v2):
#   0..511     labeled samples        (128*cls + i)
#   512..1023  s samples              (512 + 128*cls + i)   group g = cls
#   1024..1535 fp samples             (1024 + 128*cls + i)  group g = 4+cls
#   1536..1539 l-means, 1540..1543 s-means, 1544..1547 fp-means
#   1584 guaranteed-zero row, 1600 dump row
R = 1664  # padded M rows (13 * 128)
MEANB = 1536  # means block base
EARLY_ROWS = 1600  # early AllReduce covers [0, 1600)
LATE_LO, LATE_HI = 1536, 1552  # late AllReduce region
ZROW = 1584  # guaranteed-zero row (pad anchors)
DROW = 1600  # dump row for pad scatter slots
GSLOTS = 256  # gather slots per tensor per core (2 tiles of 128)
ANCH = 128  # anchor slots per core
SAPC = 64  # sample anchors per core (512 / 8)
MEANSLOT = 64  # mean anchors occupy slots 64..67 on every core
PTOT = 96  # anchor partitions actually used (64 sample + 32 mean-block)

_BUILT = None  # cached compiled Bass module
_TRACE = False  # test harness can flip this for profiling
_LAST_RESULTS = None  # test harness introspection
_LAST_IN_MAPS = None  # test harness introspection


# ----------------------------------------------------------------------------
# Host-side bit-exact replication of the reference's randomness / sampling
# ----------------------------------------------------------------------------

def _threefry2x32_pair(k0, k1, x0, x1):
    """Elementwise jax threefry2x32 block cipher (partitionable layout)."""
    x0 = x0.astype(np.uint32).copy()
    x1 = x1.astype(np.uint32).copy()
    rotations = [[13, 15, 26, 6], [17, 29, 16, 24]]
    ks = [np.uint32(k0), np.uint32(k1),
          np.uint32(np.uint32(k0) ^ np.uint32(k1) ^ np.uint32(0x1BD11BDA))]

    def rotl(x, d):
        return ((x << np.uint32(d)) | (x >> np.uint32(32 - d))).astype(np.uint32)

    x0 = (x0 + ks[0]).astype(np.uint32)
    x1 = (x1 + ks[1]).astype(np.uint32)
    for i in range(5):
        for r in rotations[i % 2]:
            x0 = (x0 + x1).astype(np.uint32)
            x1 = rotl(x1, r)
            x1 = (x0 ^ x1).astype(np.uint32)
        x0 = (x0 + ks[(i + 1) % 3]).astype(np.uint32)
        x1 = (x1 + ks[(i + 2) % 3] + np.uint32(i + 1)).astype(np.uint32)
    return x0, x1


def _np_split(k, n):
    b1, b2 = _threefry2x32_pair(k[0], k[1], np.zeros(n, np.uint32),
                                np.arange(n, dtype=np.uint32))
    return np.stack([b1, b2], axis=1)


def _np_uniform(k, n):
    b1, b2 = _threefry2x32_pair(k[0], k[1], np.zeros(n, np.uint32),
                                np.arange(n, dtype=np.uint32))
    bits = (b1 ^ b2).astype(np.uint32)
    fb = (bits >> np.uint32(9)) | np.uint32(0x3F800000)
    return fb.view(np.float32) - np.float32(1.0)


def _sample_idx(mask, n, key):
    """Index selection of reference._sample_feats: (global idx [n], nv)."""
    nv = int(mask.sum())
    order = np.argsort(np.where(mask, 0, 1).astype(np.int32), kind="stable")
    u = _np_uniform(key, n)
    rand_idx = np.floor(u * np.float32(nv)).astype(np.int32)
    rep_idx = (np.arange(n) % max(nv, 1)).astype(np.int32)
    idx = rand_idx if nv >= n else rep_idx
    return order[idx], nv


def _rank_of_valid(mask, key):
    r = _np_uniform(key, mask.shape[0])
    keys = np.where(mask, r, np.float32(2.0)).astype(np.float32)
    order = np.argsort(keys, kind="stable")
    ranks = np.empty_like(order)
    ranks[order] = np.arange(order.shape[0])
    return ranks


def _control_path(pred_gt, logits_u, label_u):
    pred_flat = pred_gt.reshape(N)
    lab_u_flat = label_u.reshape(N)
    log_u_flat = logits_u.reshape(N)
    thr = np.float32(np.mean(log_u_flat, dtype=np.float64))

    key = np.array([0, 42], np.uint32)
    classes = []
    for cls in range(NUM_CLASS):
        parts = _np_split(key, 5)
        key, k1, k2, k3, k4 = parts[0], parts[1], parts[2], parts[3], parts[4]
        ml = pred_flat == cls
        idx_l, nv_l = _sample_idx(ml, NSAMP, k1)
        mu = (lab_u_flat == cls) & (log_u_flat >= thr)
        ranks = _rank_of_valid(mu, k2)
        half = int(mu.sum()) // 2
        ms = mu & (ranks < half)
        mfp = mu & (ranks >= half)
        idx_s, nv_s = _sample_idx(ms, NSAMP, k3)
        idx_fp, nv_fp = _sample_idx(mfp, NSAMP, k4)
        classes.append({
            "l": (idx_l, nv_l, ml),
            "s": (idx_s, nv_s, ms),
            "fp": (idx_fp, nv_fp, mfp),
        })
    return thr, classes


# ----------------------------------------------------------------------------
# Pure-host fallback (degenerate masks / gather overflow; never hit on the
# benchmark distribution)
# ----------------------------------------------------------------------------

def _host_reference(inp, classes):
    f = {
        "l": inp["feat_x"].transpose(0, 2, 3, 1).reshape(N, D),
        "s": inp["feat_u_s"].transpose(0, 2, 3, 1).reshape(N, D),
        "fp": inp["feat_u_fp"].transpose(0, 2, 3, 1).reshape(N, D),
    }
    feats_l, val_l, lab_l, feats_u, val_u, labs_u = [], [], [], [], [], []
    for cls in range(NUM_CLASS):
        for t in ("l", "s", "fp"):
            idx, nv, mask = classes[cls][t]
            sampled = f[t][idx]
            mean = (f[t][mask].sum(0, dtype=np.float64) / max(nv, 1)).astype(np.float32)
            feats = np.concatenate([sampled, mean[None]], 0)
            valid = np.full(NSAMP + 1, nv > 0)
            if t == "l":
                feats_l.append(feats); val_l.append(valid)
                lab_l.append(np.full(NSAMP + 1, cls))
            else:
                feats_u.append(feats); val_u.append(valid)
                labs_u.append(np.full(NSAMP + 1, cls))
    feat_l = np.concatenate(feats_l).astype(np.float64)
    feat_u = np.concatenate(feats_u).astype(np.float64)
    val_l = np.concatenate(val_l); val_u = np.concatenate(val_u)
    lab_l = np.concatenate(lab_l); labs_u = np.concatenate(labs_u)
    if not (val_l.any() and val_u.any()):
        return np.float32(0.0)
    logits = feat_l @ feat_u.T / TEMP
    logits = np.where(val_u[None, :], logits, -1e9)
    logits = logits - logits.max(1, keepdims=True)
    log_denom = np.log(np.exp(logits).sum(1, keepdims=True))
    log_prob = np.where(val_u[None, :], logits - log_denom, 0.0)
    pos = ((lab_l[:, None] == labs_u[None, :]) & val_u[None, :]).astype(np.float64)
    mean_lpp = (pos * log_prob).sum(1) / (pos.sum(1) + 1e-12)
    loss = -(mean_lpp * val_l).sum() / max(val_l.sum(), 1)
    return np.float32(loss)


# ----------------------------------------------------------------------------
# Bass program (SPMD, identical on all 8 cores)
# ----------------------------------------------------------------------------

def _build_bass():
    global _BUILT
    if _BUILT is not None:
        return _BUILT

    import concourse.bacc as bacc
    import concourse.bass as bass
    import concourse.mybir as mybir
    import concourse.tile as tile
    from concourse.masks import make_identity

    F32 = mybir.dt.float32
    F32R = mybir.dt.float32r
    BF16 = mybir.dt.bfloat16
    I32 = mybir.dt.int32
    AX = mybir.AxisListType.X
    ALU = mybir.AluOpType
    ACT = mybir.ActivationFunctionType

    nc = bacc.Bacc("TRN2", target_bir_lowering=False, debug=False,
                   enable_asserts=False, num_devices=8)

    feats = [nc.dram_tensor(nm, [NPIX, D], F32, kind="ExternalInput")
             for nm in ("fl", "fs", "ffp")]
    msks = [nc.dram_tensor(nm, [128, 4 * (NPIX // 128)], BF16, kind="ExternalInput")
            for nm in ("mkl", "mks", "mkfp")]
    nvinv = nc.dram_tensor("nvinv", [4, 3], F32, kind="ExternalInput")
    gsrc = nc.dram_tensor("gsrc", [3 * GSLOTS, 1], I32, kind="ExternalInput")
    gdst = nc.dram_tensor("gdst", [3 * GSLOTS, 1], I32, kind="ExternalInput")
    aidx = nc.dram_tensor("aidx", [ANCH, 1], I32, kind="ExternalInput")
    wvec = nc.dram_tensor("wvec", [ANCH, 1], F32, kind="ExternalInput")
    sel8 = nc.dram_tensor("sel8", [ANCH, 8], F32, kind="ExternalInput")
    invnp = nc.dram_tensor("invnp", [ANCH, 1], F32, kind="ExternalInput")
    wvec2 = nc.dram_tensor("wvec2", [32, 1], F32, kind="ExternalInput")
    sel82 = nc.dram_tensor("sel82", [32, 8], F32, kind="ExternalInput")
    invnp2 = nc.dram_tensor("invnp2", [32, 1], F32, kind="ExternalInput")
    chain = nc.dram_tensor("chain", [1, 1], F32, kind="ExternalInput")
    out = nc.dram_tensor("out", [1, 1], F32, kind="ExternalOutput")

    NBLK = 4   # 2MB feature blocks per tensor
    CPB = 32   # 128-pixel chunks per block
    NCHUNK = NBLK * CPB

    with tile.TileContext(nc) as tc:
        with (
            tc.tile_pool(name="dram", bufs=1, space="DRAM") as dpool,
            tc.tile_pool(name="feat", bufs=6) as featp,
            tc.tile_pool(name="stat", bufs=1) as statp,
            tc.tile_pool(name="gath", bufs=3) as gathp,
            tc.tile_pool(name="tail", bufs=1) as tailp,
            tc.tile_pool(name="psA", bufs=1, space="PSUM") as psA,
            tc.tile_pool(name="psB", bufs=1, space="PSUM") as psB,
        ):
            mcon = dpool.tile([R, D], F32, name="mcon")
            mred = dpool.tile([R, D], F32, name="mred")

            # --- static setup -------------------------------------------------
            zt = statp.tile([128, R], F32, name="zt", tag="zt")
            nc.gpsimd.memset(zt[:], 0.0)
            nc.sync.dma_start(
                out=mcon.rearrange("(p a) d -> p (a d)", p=128), in_=zt[:]
            )
            mtiles = []
            for t in range(3):
                mt = statp.tile([128, 4 * NCHUNK], BF16, name=f"mt{t}", tag=f"mt{t}")
                nc.sync.dma_start(out=mt[:], in_=msks[t][:, :])
                mtiles.append(mt)
            nvt = statp.tile([4, 3], F32, name="nvt", tag="nvt")
            nc.sync.dma_start(out=nvt[:], in_=nvinv[:, :])
            ident = statp.tile([128, 128], F32, name="ident", tag="ident")
            make_identity(nc, ident[:])
            av = statp.tile([128, 1], I32, name="av", tag="av")
            nc.sync.dma_start(out=av[:], in_=aidx[:, :])
            se8 = statp.tile([128, 8], F32, name="se8", tag="se8")
            nc.sync.dma_start(out=se8[:], in_=sel8[:, :])
            inp1 = statp.tile([128, 1], F32, name="inp1", tag="inp1")
            nc.sync.dma_start(out=inp1[:], in_=invnp[:, :])
            wv = statp.tile([128, 1], F32, name="wv", tag="wv")
            nc.sync.dma_start(out=wv[:], in_=wvec[:, :])
            se82t = statp.tile([32, 8], F32, name="se82t", tag="se82t")
            nc.sync.dma_start(out=se82t[:], in_=sel82[:, :])
            inp2t = statp.tile([32, 1], F32, name="inp2t", tag="inp2t")
            nc.sync.dma_start(out=inp2t[:], in_=invnp2[:, :])
            wv2t = statp.tile([32, 1], F32, name="wv2t", tag="wv2t")
            nc.sync.dma_start(out=wv2t[:], in_=wvec2[:, :])
            cht = statp.tile([1, 1], F32, name="cht", tag="cht")
            nc.sync.dma_start(out=cht[:], in_=chain[:, :])
            onesv = statp.tile([128, 1], F32, name="onesv", tag="onesv")
            nc.gpsimd.memset(onesv[:], 1.0)
            ltm = statp.tile([128, 32], F32R, name="ltm", tag="ltm")
            nc.scalar.mul(ltm[:], ident[:, 0:32], 0.0)

            # --- sampled-row gathers -> scatter into mcon ---------------------
            for t in range(3):
                for hhalf in range(2):
                    g = 2 * t + hhalf
                    sidx = statp.tile([128, 1], I32, name=f"sidx{g}", tag=f"sidx{g}")
                    didx = statp.tile([128, 1], I32, name=f"didx{g}", tag=f"didx{g}")
                    nc.sync.dma_start(out=sidx[:], in_=gsrc[128 * g:128 * (g + 1), :])
                    nc.sync.dma_start(out=didx[:], in_=gdst[128 * g:128 * (g + 1), :])
                    grows = gathp.tile([128, D], F32, name=f"grows{g}", tag="grows")
                    nc.gpsimd.indirect_dma_start(
                        out=grows[:], out_offset=None,
                        in_=feats[t][:, :],
                        in_offset=bass.IndirectOffsetOnAxis(ap=sidx[:, :1], axis=0),
                    )
                    nc.gpsimd.indirect_dma_start(
                        out=mcon[:, :],
                        out_offset=bass.IndirectOffsetOnAxis(ap=didx[:, :1], axis=0),
                        in_=grows[:], in_offset=None,
                    )

            # --- EARLY AllReduce: sampled rows (runs under the stream) --------
            nc.gpsimd.collective_compute(
                "AllReduce", ALU.add, replica_groups=[list(range(8))],
                ins=[mcon[0:EARLY_ROWS, :].opt()],
                outs=[mred[0:EARLY_ROWS, :].opt()],
            )

            # --- U^T sample columns + anchor matrix (overlap the stream) ------
            ut = tailp.tile([128, NU], F32R, name="ut", tag="ut")
            for j in range(8):
                mr = gathp.tile([128, D], F32, name=f"mr{j}", tag="grows")
                nc.sync.dma_start(
                    out=mr[:], in_=mred[512 + 128 * j:512 + 128 * (j + 1), :]
                )
                pst = psA.tile([128, 128], F32, name=f"pst{j}", tag="psa")
                nc.tensor.transpose(pst[:, :], mr[:], ident[:])
                nc.vector.tensor_copy(out=ut[:, 128 * j:128 * (j + 1)], in_=pst[:, :])
            arows = gathp.tile([128, D], F32, name="arows", tag="grows")
            nc.gpsimd.indirect_dma_start(
                out=arows[:], out_offset=None,
                in_=mred[:, :],
                in_offset=bass.IndirectOffsetOnAxis(ap=av[:, :1], axis=0),
            )
            psl = psA.tile([128, 128], F32, name="psl", tag="psa")
            nc.tensor.transpose(psl[:, :], arows[:], ident[:])
            lt = tailp.tile([128, 128], F32R, name="lt", tag="lt")
            nc.scalar.mul(lt[:], psl[:, :], 1.0 / TEMP)

            # sample-anchor logits over sample contrasts (cols 0..1023)
            plog = psB.tile([SAPC, NU], F32, name="plog", tag="psb")
            for s0, s1 in ((0, 512), (512, 1024)):
                nc.tensor.matmul(
                    plog[:, s0:s1], lhsT=lt[:, 0:SAPC],
                    rhs=ut[:, s0:s1], start=True, stop=True,
                )
            negm = tailp.tile([SAPC, 1], F32, name="negm", tag="negm")
            escr = tailp.tile([SAPC, NU], F32, name="escr", tag="escr")
            sacc = tailp.tile([SAPC, 1], F32, name="sacc", tag="sacc")
            r8 = tailp.tile([SAPC, 8], F32, name="r8", tag="r8")
            nc.vector.reduce_max(negm[:], plog[:, 0:1024], axis=AX, negate=True)
            nc.scalar.activation(
                out=escr[:, 0:1024], in_=plog[:, 0:1024], func=ACT.Exp,
                bias=negm[:], scale=1.0, accum_out=sacc[:],
            )
            nc.vector.reduce_sum(
                r8[:], plog[:, 0:1024].rearrange("p (g x) -> p g x", g=8), axis=AX
            )

            # --- masked-sum stream (3 x 8MB, PE f32r chunk matmuls) -----------
            psmean = psA.tile([4, 3 * D], F32, name="psmean", tag="psmean")
            for b in range(NBLK):
                for t in range(3):
                    ft = featp.tile([128, CPB * D], BF16, name=f"ft{b}_{t}", tag="feat")
                    src = feats[t][4096 * b:4096 * (b + 1), :]
                    nc.gpsimd.dma_start(
                        out=ft[:],
                        in_=src.rearrange("(q k) d -> q (k d)", q=128),
                    )
                    for kk in range(CPB):
                        c = CPB * b + kk
                        nc.tensor.matmul(
                            psmean[:, D * t:D * (t + 1)],
                            lhsT=mtiles[t][:, 4 * c:4 * c + 4],
                            rhs=ft[:, D * kk:D * (kk + 1)],
                            start=(b == 0 and kk == 0),
                            stop=(b == NBLK - 1 and kk == CPB - 1),
                        )

            # --- write the 12 partial mean rows, LATE AllReduce ---------------
            for t in range(3):
                mst = tailp.tile([4, D], F32, name=f"mst{t}", tag=f"mst{t}")
                nc.scalar.mul(mst[:], psmean[:, D * t:D * (t + 1)], nvt[:, t:t + 1])
                nc.sync.dma_start(
                    out=mcon[MEANB + 4 * t:MEANB + 4 * (t + 1), :], in_=mst[:]
                )
            nc.gpsimd.collective_compute(
                "AllReduce", ALU.add, replica_groups=[list(range(8))],
                ins=[mcon[LATE_LO:LATE_HI, :].opt()],
                outs=[mred[LATE_LO:LATE_HI, :].opt()],
            )

            # --- tail: means into U^T cols + mean-anchor block ----------------
            mrm = gathp.tile([128, D], F32, name="mrm", tag="grows")
            nc.sync.dma_start(out=mrm[:], in_=mred[MEANB:MEANB + 128, :])
            psm = psA.tile([128, 128], F32, name="psm", tag="psa")
            nc.tensor.transpose(psm[:, :], mrm[:], ident[:])
            nc.vector.tensor_copy(out=ut[:, 1024:1032], in_=psm[:, 4:12])
            nc.scalar.mul(ltm[:, 0:4], psm[:, 0:4], 1.0 / TEMP)

            nc.tensor.matmul(plog[:, 1024:1032], lhsT=lt[:, 0:SAPC],
                             rhs=ut[:, 1024:1032], start=True, stop=True)
            plog2 = psB.tile([32, NU], F32, name="plog2", tag="psb2")
            for s0, s1 in ((0, 512), (512, 1024), (1024, NU)):
                nc.tensor.matmul(plog2[:, s0:s1], lhsT=ltm[:],
                                 rhs=ut[:, s0:s1], start=True, stop=True)

            # sample-anchor fixups (mean columns; all groups valid by host gate)
            e8m = tailp.tile([SAPC, 8], F32, name="e8m", tag="e8m")
            sacc2 = tailp.tile([SAPC, 1], F32, name="sacc2", tag="sacc2")
            nc.scalar.activation(out=e8m[:], in_=plog[:, 1024:1032],
                                 func=ACT.Exp, bias=negm[:], scale=1.0,
                                 accum_out=sacc2[:])
            r8m = tailp.tile([SAPC, 8], F32, name="r8m", tag="r8m")
            nc.vector.tensor_copy(out=r8m[:], in_=plog[:, 1024:1032])
            sg = tailp.tile([SAPC, 8], F32, name="sg", tag="sg")
            nc.vector.tensor_tensor(out=sg[:], in0=r8[:], in1=r8m[:], op=ALU.add)
            ssum = tailp.tile([SAPC, 1], F32, name="ssum", tag="ssum")
            nc.vector.tensor_tensor(out=ssum[:], in0=sacc[:], in1=sacc2[:], op=ALU.add)
            lns = tailp.tile([SAPC, 1], F32, name="lns", tag="lns")
            nc.scalar.activation(out=lns[:], in_=ssum[:], func=ACT.Ln)
            junk8b = tailp.tile([SAPC, 8], F32, name="junk8b", tag="junk8b")
            spos = tailp.tile([SAPC, 1], F32, name="spos", tag="spos")
            nc.vector.tensor_tensor(out=junk8b[:], in0=sg[:], in1=se8[0:SAPC, :],
                                    op=ALU.mult)
            nc.vector.reduce_sum(spos[:], junk8b[:], axis=AX)
            t1 = tailp.tile([SAPC, 1], F32, name="t1", tag="t1")
            nc.vector.tensor_tensor(out=t1[:], in0=spos[:], in1=inp1[0:SAPC, :],
                                    op=ALU.mult)
            nc.vector.tensor_tensor(out=t1[:], in0=t1[:], in1=negm[:], op=ALU.add)
            nc.vector.tensor_tensor(out=t1[:], in0=t1[:], in1=lns[:], op=ALU.subtract)
            nc.vector.tensor_tensor(out=t1[:], in0=t1[:], in1=wv[0:SAPC, :],
                                    op=ALU.mult)

            # mean-anchor block mini-softmax (rows on partitions 0..31)
            negm2 = tailp.tile([32, 1], F32, name="negm2", tag="negm2")
            nc.vector.reduce_max(negm2[:], plog2[:, 0:1024], axis=AX, negate=True)
            escr2 = tailp.tile([32, NU], F32, name="escr2", tag="escr2")
            s12 = tailp.tile([32, 1], F32, name="s12", tag="s12")
            nc.scalar.activation(
                out=escr2[:, 0:1024], in_=plog2[:, 0:1024], func=ACT.Exp,
                bias=negm2[:], scale=1.0, accum_out=s12[:],
            )
            r82 = tailp.tile([32, 8], F32, name="r82", tag="r82")
            nc.vector.reduce_sum(
                r82[:], plog2[:, 0:1024].rearrange("p (g x) -> p g x", g=8), axis=AX
            )
            e8m2 = tailp.tile([32, 8], F32, name="e8m2", tag="e8m2")
            s22 = tailp.tile([32, 1], F32, name="s22", tag="s22")
            nc.scalar.activation(out=e8m2[:], in_=plog2[:, 1024:1032],
                                 func=ACT.Exp, bias=negm2[:], scale=1.0,
                                 accum_out=s22[:])
            r8m2 = tailp.tile([32, 8], F32, name="r8m2", tag="r8m2")
            nc.vector.tensor_copy(out=r8m2[:], in_=plog2[:, 1024:1032])
            sg2 = tailp.tile([32, 8], F32, name="sg2", tag="sg2")
            nc.vector.tensor_tensor(out=sg2[:], in0=r82[:], in1=r8m2[:], op=ALU.add)
            ssum2 = tailp.tile([32, 1], F32, name="ssum2", tag="ssum2")
            nc.vector.tensor_tensor(out=ssum2[:], in0=s12[:], in1=s22[:], op=ALU.add)
            lns2 = tailp.tile([32, 1], F32, name="lns2", tag="lns2")
            nc.scalar.activation(out=lns2[:], in_=ssum2[:], func=ACT.Ln)
            junk8b2 = tailp.tile([32, 8], F32, name="junk8b2", tag="junk8b2")
            spos2 = tailp.tile([32, 1], F32, name="spos2", tag="spos2")
            nc.vector.tensor_tensor(out=junk8b2[:], in0=sg2[:], in1=se82t[:],
                                    op=ALU.mult)
            nc.vector.reduce_sum(spos2[:], junk8b2[:], axis=AX)
            t1b = tailp.tile([32, 1], F32, name="t1b", tag="t1b")
            nc.vector.tensor_tensor(out=t1b[:], in0=spos2[:], in1=inp2t[:],
                                    op=ALU.mult)
            nc.vector.tensor_tensor(out=t1b[:], in0=t1b[:], in1=negm2[:], op=ALU.add)
            nc.vector.tensor_tensor(out=t1b[:], in0=t1b[:], in1=lns2[:],
                                    op=ALU.subtract)
            nc.vector.tensor_tensor(out=t1b[:], in0=t1b[:], in1=wv2t[:],
                                    op=ALU.mult)

            pssc = psA.tile([1, 1], F32, name="pssc", tag="psa")
            nc.tensor.matmul(pssc[:, :], lhsT=t1[:], rhs=onesv[0:SAPC, :],
                             start=True, stop=False)
            nc.tensor.matmul(pssc[:, :], lhsT=t1b[:], rhs=onesv[0:32, :],
                             start=False, stop=True)
            osb = tailp.tile([1, 1], F32, name="osb", tag="osb")
            nc.vector.tensor_tensor(out=osb[:], in0=pssc[:, :], in1=cht[:],
                                    op=ALU.add)
            nc.sync.dma_start(out=out[:, :], in_=osb[:])

    nc.compile()
    _BUILT = nc
    return nc


# ------------------------------------------------------------------------------------
# Host driver
# ----------------------------------------------------------------------------

def _prep_core_inputs(inp, thr, classes):
    """Builds the 8 per-core input dicts (numpy), v2 physical M layout."""
    fT = {
        "fl": np.ascontiguousarray(
            inp["feat_x"].transpose(0, 2, 3, 1).reshape(B, NPIX, D)),
        "fs": np.ascontiguousarray(
            inp["feat_u_s"].transpose(0, 2, 3, 1).reshape(B, NPIX, D)),
        "ffp": np.ascontiguousarray(
            inp["feat_u_fp"].transpose(0, 2, 3, 1).reshape(B, NPIX, D)),
    }
    tkeys = ["l", "s", "fp"]

    # one-hot mask channels (exact in bf16); per-(class, tensor) 1/nv scales
    import ml_dtypes
    mvals = np.zeros((3, N, 4), np.float32)
    nvinv = np.zeros((4, 3), np.float32)
    for cls in range(NUM_CLASS):
        for t in range(3):
            idx, nv, mask = classes[cls][tkeys[t]]
            mvals[t, mask, cls] = np.float32(1.0)
            nvinv[cls, t] = np.float32(1.0) / np.float32(max(nv, 1))

    # device mask layout per core: [128, 4*128]; column 4*c+m, partition q
    # maps to local pixel 4096*(c//32) + 32*q + (c%32)
    mdev = np.zeros((3, 8, 128, 4 * 128), ml_dtypes.bfloat16)
    for t in range(3):
        percore = mvals[t].reshape(8, NPIX, 4)
        x = percore.reshape(8, 4, 128, 32, 4)  # [core, b, q, k, m]
        x = x.transpose(0, 2, 1, 3, 4).reshape(8, 128, 512)
        mdev[t] = x.astype(ml_dtypes.bfloat16)

    # gather/scatter slot lists (physical sample rows)
    gsrc = np.zeros((8, 3 * GSLOTS, 1), np.int32)
    gdst = np.full((8, 3 * GSLOTS, 1), DROW, np.int32)
    counts = np.zeros((8, 3), np.int32)
    for t in range(3):
        for cls in range(NUM_CLASS):
            idx, nv, mask = classes[cls][tkeys[t]]
            dstbase = 512 * t + 128 * cls
            owner = idx >> 14
            local = idx & (NPIX - 1)
            for i in range(NSAMP):
                c = owner[i]
                s = counts[c, t]
                if s >= GSLOTS:
                    return None  # overflow -> host fallback
                gsrc[c, GSLOTS * t + s, 0] = local[i]
                gdst[c, GSLOTS * t + s, 0] = dstbase + i
                counts[c, t] += 1

    # validity: val_lg[cls] for labeled groups; val_g[g] for contrast groups
    # with v2 group order g = cls (s) and 4+cls (fp)
    val_g = np.zeros(8, bool)
    val_lg = np.zeros(NUM_CLASS, bool)
    for cls in range(NUM_CLASS):
        val_lg[cls] = classes[cls]["l"][1] > 0
        val_g[cls] = classes[cls]["s"][1] > 0
        val_g[4 + cls] = classes[cls]["fp"][1] > 0

    val_l_total = float(val_lg.sum() * GRP)
    wscale = np.float32(-1.0) / np.float32(max(val_l_total, 1.0))

    if not (val_lg.all() and val_g.all()):
        return None  # device program assumes all groups valid -> host fallback

    aidx = np.full((8, ANCH, 1), ZROW, np.int32)
    wv = np.zeros((8, ANCH, 1), np.float32)
    se8 = np.zeros((8, ANCH, 8), np.float32)
    inp1 = np.zeros((8, ANCH, 1), np.float32)

    def anchor_params(cls):
        sel = np.zeros(8, np.float32)
        for g in (cls, 4 + cls):
            if val_g[g]:
                sel[g] = 1.0
        npos = np.float32(sel.sum() * GRP)
        invn = np.float32(1.0) / (npos + np.float32(1e-12))
        return sel, invn

    for c in range(8):
        # sample anchors: slots 0..63 <- physical rows [64c, 64c+64)
        for s in range(SAPC):
            row = SAPC * c + s
            cls = row // 128
            aidx[c, s, 0] = row
            if val_lg[cls]:
                wv[c, s, 0] = wscale
            sel, invn = anchor_params(cls)
            se8[c, s] = sel
            inp1[c, s, 0] = invn

    # mean-anchor block (partitions 0..31 of plog2; weighted on core 0 only)
    wv2 = np.zeros((8, 32, 1), np.float32)
    se82 = np.zeros((8, 32, 8), np.float32)
    inp2 = np.zeros((8, 32, 1), np.float32)
    for cls in range(NUM_CLASS):
        sel, invn = anchor_params(cls)
        for c in range(8):
            se82[c, cls] = sel
            inp2[c, cls, 0] = invn
        if val_lg[cls]:
            wv2[0, cls, 0] = wscale

    in_maps = []
    for c in range(8):
        in_maps.append({
            "fl": fT["fl"][c],
            "fs": fT["fs"][c],
            "ffp": fT["ffp"][c],
            "mkl": mdev[0, c],
            "mks": mdev[1, c],
            "mkfp": mdev[2, c],
            "nvinv": nvinv,
            "gsrc": gsrc[c],
            "gdst": gdst[c],
            "aidx": aidx[c],
            "wvec": wv[c],
            "sel8": se8[c],
            "invnp": inp1[c],
            "wvec2": wv2[c],
            "sel82": se82[c],
            "invnp2": inp2[c],
            "chain": np.zeros((1, 1), np.float32),
        })
    return in_maps, val_lg, val_g


def kernel(**inputs):
    global _LAST_RESULTS, _LAST_IN_MAPS
    inp = {k: np.ascontiguousarray(np.asarray(v)) for k, v in inputs.items()}
    thr, classes = _control_path(inp["pred_gt"], inp["logits_u"], inp["label_u"])

    prep = _prep_core_inputs(inp, thr, classes)
    if prep is None:
        return np.array(_host_reference(inp, classes), dtype=np.float32)
    in_maps, val_lg, val_g = prep
    if not (val_lg.any() and val_g.any()):
        return np.array(np.float32(0.0), dtype=np.float32)

    from concourse import bass_utils

    nc = _build_bass()
    res = bass_utils.run_bass_kernel_spmd(
        nc, in_maps, core_ids=list(range(8)),
        trace=_TRACE, stitch_traces=_TRACE,
    )
    _LAST_RESULTS = res
    _LAST_IN_MAPS = in_maps
    loss = np.float64(0.0)
    for c in range(8):
        loss += np.float64(res.results[c]["out"][0, 0])
    return np.array(np.float32(loss), dtype=np.float32)

